# revision 17
# baseline (speedup 1.0000x reference)
"""Block-sparse (banded) attention kernel for Trainium2, 8 NeuronCores.

Sharding: data-parallel over batch (2) x tensor-parallel over heads
(16 heads -> 4 per core).  Each core computes its 4 heads' Q/K/V
projections, banded block attention (|r-c| <= 15 blocks, per-block
softmax), and a partial output projection; the host sums the 4 partial
outputs per batch element.

bf16 matmul pipeline (1 PE cycle/row), fp32 PSUM accumulation.  All
inputs are cast to bf16 on the host and DMA'd over HWDGE, so no device
engine spends time casting.  Attention is software-pipelined at the
column-block-pair level: unit i's QK/softmax-denominator matmuls
interleave with unit i-1's broadcast/AV matmuls, with 2-slot lags on
the cross-engine (exp, rescale) dependencies, so the in-order PE queue
never stalls and the clock stays at the high p-state.

Self-contained: hardcodes all shapes; only needs the concourse tree that
the environment already puts on sys.path.
"""

import sys

for _p in ("/opt/trn_rl_repo",):
    if _p not in sys.path:
        sys.path.insert(0, _p)

from contextlib import ExitStack

import ml_dtypes
import numpy as np

import concourse.bacc as bacc
import concourse.tile as tile
from concourse import bass_utils, mybir

F32 = mybir.dt.float32
BF16 = mybir.dt.bfloat16
EXP = mybir.ActivationFunctionType.Exp
BF16NP = ml_dtypes.bfloat16

B, S, E = 2, 2048, 1024
H, HD, BLK = 16, 64, 64
NB = S // BLK  # 32 blocks
NCORES = 8
HPC = 4  # heads per core
F = HPC * HD  # 256 local features
BAND = 15
SCALE = HD ** -0.5

# per r8-slab (8 query blocks, q=512) column-block ranges, even-extended
T_SLABS = 4
QS = 512  # q extent per slab
LO = []
NP_T = []
for _t in range(T_SLABS):
    lo = max(0, 8 * _t - BAND)
    hi = min(NB - 1, 8 * _t + 7 + BAND)
    if (hi - lo + 1) % 2 == 1:
        if lo > 0:
            lo -= 1
        else:
            hi += 1
    LO.append(lo)
    NP_T.append((hi - lo + 1) // 2)
MAXP = max(NP_T)  # 16 pairs

DLAG = 3  # slots between QK/exp and the denominator matmul
AVLAG = 2  # slots between broadcast/rescale and the AV matmul


class _Unit:
    __slots__ = ("t", "h", "npt", "lo", "expS", "accs", "rc", "acco", "half",
                 "pts", "nd")


def build_nc(debug=False):
    nc = bacc.Bacc("TRN2", target_bir_lowering=False, debug=False)

    xq_d = nc.dram_tensor("xqT", [E, S], BF16, kind="ExternalInput")
    xk_d = nc.dram_tensor("xkT", [E, S], BF16, kind="ExternalInput")
    xv_d = nc.dram_tensor("xvT", [E, S], BF16, kind="ExternalInput")
    wq_d = nc.dram_tensor("wqT", [E, F], BF16, kind="ExternalInput")
    wk_d = nc.dram_tensor("wkT", [E, F], BF16, kind="ExternalInput")
    wv_d = nc.dram_tensor("wvT", [E, F], BF16, kind="ExternalInput")
    wo_d = nc.dram_tensor("woT", [F, E], BF16, kind="ExternalInput")
    sel_d = nc.dram_tensor("selc", [128, MAXP * 32], BF16, kind="ExternalInput")
    bds_d = nc.dram_tensor("bdsel", [32, MAXP * 128], BF16, kind="ExternalInput")
    vm_d = nc.dram_tensor("vmask", [32, T_SLABS * QS], F32, kind="ExternalInput")
    out_d = nc.dram_tensor("out", [S, E], F32, kind="ExternalOutput")

    with tile.TileContext(nc) as tc, ExitStack() as ctx, nc.allow_low_precision(
        reason="bf16 matmul pipeline; fp32 PSUM accumulation"
    ):
        pers = ctx.enter_context(tc.tile_pool(name="pers", bufs=1))
        qT = pers.tile([128, 2 * S], BF16, tag="qT")
        kT = pers.tile([128, 2 * S], BF16, tag="kT")
        vv = pers.tile([128, 16 * F], BF16, tag="vv")
        xva = pers.tile([128, 8 * 2048], BF16, tag="xva")
        wq = pers.tile([128, 8 * F], BF16, tag="wq")
        wk = pers.tile([128, 8 * F], BF16, tag="wk")
        wv = pers.tile([128, 8 * F], BF16, tag="wv")
        wo = pers.tile([128, 2 * E], BF16, tag="wo")
        selb = pers.tile([128, MAXP * 32], BF16, tag="selb")
        bds = pers.tile([32, MAXP * 128], BF16, tag="bds")
        vm = pers.tile([32, T_SLABS * QS], F32, tag="vm")

        # k-projection weights first: phase 1 is on the critical path
        nc.sync.dma_start(
            wk[:].rearrange("p (c f) -> p c f", c=8),
            wk_d.ap().rearrange("(c p) f -> p c f", p=128),
        )

        # ---- phase 1: k projection (kT layout [f, s]) ----
        with tc.tile_pool(name="xk", bufs=3) as xkp, tc.tile_pool(
            name="psK", bufs=1, space="PSUM"
        ) as pskp:
            psK = pskp.tile([128, 4096], F32)
            for e in range(8):
                xt = xkp.tile([128, S], BF16, tag="xk")
                nc.sync.dma_start(xt[:], xk_d.ap()[e * 128 : (e + 1) * 128, :])
                for fold in range(2):
                    for sc in range(4):
                        nc.tensor.matmul(
                            psK[:, (fold * 4 + sc) * 512 : (fold * 4 + sc + 1) * 512],
                            wk[:, e * F + fold * 128 : e * F + fold * 128 + 128],
                            xt[:, sc * 512 : (sc + 1) * 512],
                            start=(e == 0),
                            stop=(e == 7),
                        )
            for fold in range(2):
                for sc in range(4):
                    nc.scalar.copy(
                        kT[:, fold * S + sc * 512 : fold * S + (sc + 1) * 512],
                        psK[:, (fold * 4 + sc) * 512 : (fold * 4 + sc + 1) * 512],
                    )

        # remaining weights/constants + resident xv after the phase-1 loads
        nc.sync.dma_start(
            wv[:].rearrange("p (c f) -> p c f", c=8),
            wv_d.ap().rearrange("(c p) f -> p c f", p=128),
        )
        nc.sync.dma_start(
            wq[:].rearrange("p (c f) -> p c f", c=8),
            wq_d.ap().rearrange("(c p) f -> p c f", p=128),
        )
        nc.sync.dma_start(selb[:], sel_d.ap())
        nc.sync.dma_start(bds[:], bds_d.ap())
        nc.sync.dma_start(vm[:], vm_d.ap())
        nc.sync.dma_start(
            wo[:].rearrange("p (c e) -> p c e", c=2),
            wo_d.ap().rearrange("(c p) e -> p c e", p=128),
        )
        for e in range(8):
            nc.sync.dma_start(
                xva[:, e * 2048 : (e + 1) * 2048],
                xv_d.ap()[e * 128 : (e + 1) * 128, :],
            )

        # ---- phase 2: v projection (natural layout [s, f]) ----
        with tc.tile_pool(name="psV", bufs=2, space="PSUM") as psvp:
            for sc in range(4):
                pvs = [
                    psvp.tile([128, 256], F32, name=f"pv{sub}", tag=f"psV{sub}")
                    for sub in range(4)
                ]
                for e in range(8):
                    for sub in range(4):
                        nc.tensor.matmul(
                            pvs[sub][:],
                            xva[
                                :,
                                e * 2048 + sc * 512 + sub * 128 : e * 2048
                                + sc * 512
                                + (sub + 1) * 128,
                            ],
                            wv[:, e * F : (e + 1) * F],
                            start=(e == 0),
                            stop=(e == 7),
                        )
                for sub in range(4):
                    nc.scalar.copy(
                        vv[:, sc * 1024 + sub * 256 : sc * 1024 + (sub + 1) * 256],
                        pvs[sub][:],
                    )

        # ---- phase 3: q projection + attention + output projection ----
        xqp = ctx.enter_context(tc.tile_pool(name="xq", bufs=4))
        psSp = ctx.enter_context(tc.tile_pool(name="psS", bufs=6, space="PSUM"))
        flexp = ctx.enter_context(tc.tile_pool(name="flex", bufs=2, space="PSUM"))
        expp = ctx.enter_context(tc.tile_pool(name="expS", bufs=2))
        ptp = ctx.enter_context(tc.tile_pool(name="pt", bufs=4))
        rcpp = ctx.enter_context(tc.tile_pool(name="rcp", bufs=2))
        attp = ctx.enter_context(tc.tile_pool(name="att", bufs=4))
        outp = ctx.enter_context(tc.tile_pool(name="outsb", bufs=2))

        def qproj(sc4):
            pqs = [
                psSp.tile([128, 512], F32, name=f"pq{fold}", tag="psS")
                for fold in range(2)
            ]
            for e2 in range(4):
                xt = xqp.tile([128, 2, 512], BF16, tag="xq")
                nc.sync.dma_start(
                    xt[:],
                    xq_d.ap()[
                        e2 * 256 : (e2 + 1) * 256, sc4 * 512 : (sc4 + 1) * 512
                    ].rearrange("(two p) q -> p two q", p=128),
                )
                for half in range(2):
                    e = 2 * e2 + half
                    for fold in range(2):
                        nc.tensor.matmul(
                            pqs[fold][:],
                            wq[:, e * F + fold * 128 : e * F + fold * 128 + 128],
                            xt[:, half, :],
                            start=(e == 0),
                            stop=(e == 7),
                        )
            for fold in range(2):
                nc.scalar.copy(
                    qT[:, fold * S + sc4 * 512 : fold * S + (sc4 + 1) * 512],
                    pqs[fold][:],
                )

        def outproj(t, atts):
            # atts: [pair0_tile, pair1_tile], each [128, 512] bf16
            for sc2 in range(4):
                ob = outp.tile([128, 1024], F32, tag="outsb")
                for eh in range(2):
                    po = flexp.tile([128, 512], F32, tag="flex")
                    for kc in range(2):
                        nc.tensor.matmul(
                            po[:],
                            atts[kc][:, sc2 * 128 : sc2 * 128 + 128],
                            wo[:, kc * E + eh * 512 : kc * E + eh * 512 + 512],
                            start=(kc == 0),
                            stop=(kc == 1),
                        )
                    nc.vector.tensor_copy(ob[:, eh * 512 : (eh + 1) * 512], po[:])
                row = (4 * t + sc2) * 128
                nc.sync.dma_start(out_d.ap()[row : row + 128, :], ob[:])

        def emit_qk(u, j):
            fold = u.h // 2
            bp = 64 * (u.h % 2)
            c0 = u.lo + 2 * j
            ps = psSp.tile([128, 512], F32, name="ps", tag="psS")
            nc.tensor.matmul(
                ps[:],
                kT[bp : bp + 64, fold * S + c0 * 64 : fold * S + c0 * 64 + 128],
                qT[bp : bp + 64, fold * S + u.t * QS : fold * S + (u.t + 1) * QS],
                start=True,
                stop=True,
            )
            nc.scalar.activation(u.expS[:, j * QS : (j + 1) * QS], ps[:], EXP)

        def emit_denom(u, j):
            nc.tensor.matmul(
                u.accs[0:32, :],
                selb[:, j * 32 : (j + 1) * 32],
                u.expS[:, j * QS : (j + 1) * QS],
                start=(j == 0),
                stop=(j == u.npt - 1),
            )

        def emit_rcp(u):
            rc = rcpp.tile([32, 512], BF16, tag="rcp")
            rs1 = rcpp.tile([32, 512], F32, tag="rcs1")
            rs2 = rcpp.tile([32, 512], F32, tag="rcs2")
            nc.vector.reciprocal_approx_accurate(rs2[:], u.accs[0:32, :], rs1[:])
            nc.vector.tensor_mul(rc[:], rs2[:], vm[:, u.t * QS : (u.t + 1) * QS])
            u.rc = rc
            u.acco = psSp.tile([128, 512], F32, name="acco", tag="psS")

        def emit_bcast_mul(u, j):
            bt = flexp.tile([128, 512], F32, tag="flex")
            nc.tensor.matmul(
                bt[:],
                bds[0 : 2 * u.npt, j * 128 : (j + 1) * 128],
                u.rc[0 : 2 * u.npt, :],
                start=True,
                stop=True,
            )
            pt = ptp.tile([128, 512], BF16, tag="pt")
            nc.vector.tensor_mul(pt[:], u.expS[:, j * QS : (j + 1) * QS], bt[:])
            u.pts.append(pt)

        def emit_av(u, j):
            cp = u.lo // 2 + j
            nc.tensor.matmul(
                u.acco[0:64, :],
                vv[:, cp * F + u.h * 64 : cp * F + u.h * 64 + 64],
                u.pts[j][:],
                start=(j == 0),
                stop=(j == u.npt - 1),
            )

        units = [(t, h) for t in range(T_SLABS) for h in range(HPC)]
        atts_by_t = {t: [] for t in range(T_SLABS)}
        done_by_t = {t: 0 for t in range(T_SLABS)}

        def finish_unit(p):
            if p.half == 0:
                attn_t = attp.tile([128, 512], BF16, tag="att")
                atts_by_t[p.t].append(attn_t)
            else:
                attn_t = atts_by_t[p.t][p.h // 2]
            nc.scalar.copy(attn_t[64 * p.half : 64 * p.half + 64, :], p.acco[0:64, :])
            done_by_t[p.t] += 1
            if done_by_t[p.t] == HPC:
                outproj(p.t, atts_by_t[p.t])

        qproj(0)
        prev = None
        for t, h in units:
            u = _Unit()
            u.t, u.h, u.npt, u.lo, u.half = t, h, NP_T[t], LO[t], h % 2
            u.expS = expp.tile([128, MAXP * QS], BF16, tag="expS")
            u.accs = psSp.tile([128, 512], F32, name="accs", tag="psS")
            u.rc = None
            u.acco = None
            u.pts = []
            u.nd = 0
            if prev is not None:
                emit_rcp(prev)
            nj = max(u.npt, prev.npt if prev is not None else 0)
            for j in range(nj):
                if j < u.npt:
                    emit_qk(u, j)
                if prev is not None and j < prev.npt:
                    emit_bcast_mul(prev, j)
                if prev is not None and j >= AVLAG and j - AVLAG < prev.npt:
                    emit_av(prev, j - AVLAG)
                if j >= DLAG and u.nd < u.npt and u.nd == j - DLAG:
                    emit_denom(u, u.nd)
                    u.nd += 1
            if prev is not None:
                for jj in range(max(nj - AVLAG, 0), prev.npt):
                    emit_av(prev, jj)
                finish_unit(prev)
            while u.nd < u.npt:
                emit_denom(u, u.nd)
                u.nd += 1
            # prefetch next slab's q projection midway through the slab
            if h == 1 and t + 1 < T_SLABS:
                qproj(t + 1)
            prev = u

        # drain the final unit
        emit_rcp(prev)
        for j in range(prev.npt):
            emit_bcast_mul(prev, j)
            if j >= AVLAG:
                emit_av(prev, j - AVLAG)
        for jj in range(max(prev.npt - AVLAG, 0), prev.npt):
            emit_av(prev, jj)
        finish_unit(prev)

    nc.compile()
    return nc


_NC_CACHE = []


def _get_nc():
    if not _NC_CACHE:
        _NC_CACHE.append(build_nc())
    return _NC_CACHE[0]


def _host_consts():
    selc = np.zeros((128, MAXP * 32), np.float32)
    for k in range(128):
        for j in range(MAXP):
            selc[k, j * 32 + 2 * j + k // 64] = 1.0
    bdsel = np.zeros((32, MAXP * 128), np.float32)
    for j in range(MAXP):
        for p in range(128):
            bdsel[2 * j + p // 64, j * 128 + p] = 1.0
    vmask = np.zeros((32, T_SLABS * QS), np.float32)
    for t in range(T_SLABS):
        for m in range(2 * NP_T[t]):
            c = LO[t] + m
            for qb in range(QS // BLK):
                r = 8 * t + qb
                if abs(r - c) <= BAND:
                    vmask[m, t * QS + qb * 64 : t * QS + (qb + 1) * 64] = 1.0
    return selc, bdsel, vmask


def kernel(query, key, value, Wq, Wk, Wv, Wo):
    query = np.asarray(query, np.float32)
    key = np.asarray(key, np.float32)
    value = np.asarray(value, np.float32)
    Wq = np.asarray(Wq, np.float32)
    Wk = np.asarray(Wk, np.float32)
    Wv = np.asarray(Wv, np.float32)
    Wo = np.asarray(Wo, np.float32)

    nc = _get_nc()
    selc, bdsel, vmask = _host_consts()

    in_maps = []
    for c in range(NCORES):
        b, g = divmod(c, HPC)
        fs = slice(F * g, F * (g + 1))
        in_maps.append(
            {
                "xqT": np.ascontiguousarray(query[b].T).astype(BF16NP),
                "xkT": np.ascontiguousarray(key[b].T).astype(BF16NP),
                "xvT": np.ascontiguousarray(value[b].T).astype(BF16NP),
                "wqT": np.ascontiguousarray((Wq[fs, :] * SCALE).T).astype(BF16NP),
                "wkT": np.ascontiguousarray(Wk[fs, :].T).astype(BF16NP),
                "wvT": np.ascontiguousarray(Wv[fs, :].T).astype(BF16NP),
                "woT": np.ascontiguousarray(Wo[:, fs].T).astype(BF16NP),
                "selc": selc.astype(BF16NP),
                "bdsel": bdsel.astype(BF16NP),
                "vmask": vmask,
            }
        )

    res = bass_utils.run_bass_kernel_spmd(nc, in_maps, core_ids=list(range(NCORES)))
    out = np.zeros((B, S, E), np.float32)
    for c in range(NCORES):
        b = c // HPC
        out[b] += res.results[c]["out"]
    return out


# revision 18
# speedup vs baseline: 1.1709x; 1.1709x over previous
"""Block-sparse (banded) attention kernel for Trainium2, 8 NeuronCores.

Sharding: data-parallel over batch (2) x tensor-parallel over heads
(16 heads -> 4 per core).  Each core computes its 4 heads' Q/K/V
projections, banded block attention (|r-c| <= 15 blocks, per-block
softmax), and a partial output projection; the host sums the 4 partial
outputs per batch element.

bf16 matmul pipeline (1 PE cycle/row), fp32 PSUM accumulation.  All
inputs are cast to bf16 on the host and DMA'd over HWDGE, so no device
engine spends time casting.  Attention is software-pipelined at the
column-block-pair level: unit i's QK/softmax-denominator matmuls
interleave with unit i-1's broadcast/AV matmuls, with 2-slot lags on
the cross-engine (exp, rescale) dependencies, so the in-order PE queue
never stalls and the clock stays at the high p-state.

Self-contained: hardcodes all shapes; only needs the concourse tree that
the environment already puts on sys.path.
"""

import sys

for _p in ("/opt/trn_rl_repo",):
    if _p not in sys.path:
        sys.path.insert(0, _p)

from contextlib import ExitStack

import ml_dtypes
import numpy as np

import concourse.bacc as bacc
import concourse.tile as tile
from concourse import bass_utils, mybir

F32 = mybir.dt.float32
BF16 = mybir.dt.bfloat16
EXP = mybir.ActivationFunctionType.Exp
BF16NP = ml_dtypes.bfloat16

B, S, E = 2, 2048, 1024
H, HD, BLK = 16, 64, 64
NB = S // BLK  # 32 blocks
NCORES = 8
HPC = 4  # heads per core
F = HPC * HD  # 256 local features
BAND = 15
SCALE = HD ** -0.5

# per r8-slab (8 query blocks, q=512) column-block ranges, even-extended
T_SLABS = 4
QS = 512  # q extent per slab
LO = []
NP_T = []
for _t in range(T_SLABS):
    lo = max(0, 8 * _t - BAND)
    hi = min(NB - 1, 8 * _t + 7 + BAND)
    if (hi - lo + 1) % 2 == 1:
        if lo > 0:
            lo -= 1
        else:
            hi += 1
    LO.append(lo)
    NP_T.append((hi - lo + 1) // 2)
MAXP = max(NP_T)  # 16 pairs

DLAG = 2  # slots between QK/exp and the denominator matmul
AVLAG = 3  # slots between broadcast/rescale and the AV matmul


class _Unit:
    __slots__ = ("t", "h", "npt", "lo", "expS", "accs", "rc", "acco", "half",
                 "pts", "nd")


def build_nc(debug=False):
    nc = bacc.Bacc("TRN2", target_bir_lowering=False, debug=False)

    xq_d = nc.dram_tensor("xqT", [E, S], BF16, kind="ExternalInput")
    xk_d = nc.dram_tensor("xkT", [E, S], BF16, kind="ExternalInput")
    xv_d = nc.dram_tensor("xvT", [E, S], BF16, kind="ExternalInput")
    wq_d = nc.dram_tensor("wqT", [E, F], BF16, kind="ExternalInput")
    wk_d = nc.dram_tensor("wkT", [E, F], BF16, kind="ExternalInput")
    wv_d = nc.dram_tensor("wvT", [E, F], BF16, kind="ExternalInput")
    wo_d = nc.dram_tensor("woT", [F, E], BF16, kind="ExternalInput")
    sel_d = nc.dram_tensor("selc", [128, MAXP * 32], BF16, kind="ExternalInput")
    bds_d = nc.dram_tensor("bdsel", [32, MAXP * 128], BF16, kind="ExternalInput")
    vm_d = nc.dram_tensor("vmask", [32, T_SLABS * QS], F32, kind="ExternalInput")
    out_d = nc.dram_tensor("out", [S, E], F32, kind="ExternalOutput")

    with tile.TileContext(nc) as tc, ExitStack() as ctx, nc.allow_low_precision(
        reason="bf16 matmul pipeline; fp32 PSUM accumulation"
    ):
        pers = ctx.enter_context(tc.tile_pool(name="pers", bufs=1))
        qT = pers.tile([128, 2 * S], BF16, tag="qT")
        kT = pers.tile([128, 2 * S], BF16, tag="kT")
        vv = pers.tile([128, 16 * F], BF16, tag="vv")
        xva = pers.tile([128, 8 * 2048], BF16, tag="xva")
        wq = pers.tile([128, 8 * F], BF16, tag="wq")
        wk = pers.tile([128, 8 * F], BF16, tag="wk")
        wv = pers.tile([128, 8 * F], BF16, tag="wv")
        wo = pers.tile([128, 2 * E], BF16, tag="wo")
        selb = pers.tile([128, MAXP * 32], BF16, tag="selb")
        bds = pers.tile([32, MAXP * 128], BF16, tag="bds")
        vm = pers.tile([32, T_SLABS * QS], F32, tag="vm")

        # k-projection weights first: phase 1 is on the critical path
        nc.sync.dma_start(
            wk[:].rearrange("p (c f) -> p c f", c=8),
            wk_d.ap().rearrange("(c p) f -> p c f", p=128),
        )

        # ---- phase 1: k projection (kT layout [f, s]) ----
        with tc.tile_pool(name="xk", bufs=3) as xkp, tc.tile_pool(
            name="psK", bufs=1, space="PSUM"
        ) as pskp:
            psK = pskp.tile([128, 4096], F32)
            for e in range(8):
                xt = xkp.tile([128, S], BF16, tag="xk")
                nc.sync.dma_start(xt[:], xk_d.ap()[e * 128 : (e + 1) * 128, :])
                for fold in range(2):
                    for sc in range(4):
                        nc.tensor.matmul(
                            psK[:, (fold * 4 + sc) * 512 : (fold * 4 + sc + 1) * 512],
                            wk[:, e * F + fold * 128 : e * F + fold * 128 + 128],
                            xt[:, sc * 512 : (sc + 1) * 512],
                            start=(e == 0),
                            stop=(e == 7),
                        )
            for fold in range(2):
                for sc in range(4):
                    nc.scalar.copy(
                        kT[:, fold * S + sc * 512 : fold * S + (sc + 1) * 512],
                        psK[:, (fold * 4 + sc) * 512 : (fold * 4 + sc + 1) * 512],
                    )

        # remaining weights/constants + resident xv after the phase-1 loads
        nc.sync.dma_start(
            wv[:].rearrange("p (c f) -> p c f", c=8),
            wv_d.ap().rearrange("(c p) f -> p c f", p=128),
        )
        nc.sync.dma_start(
            wq[:].rearrange("p (c f) -> p c f", c=8),
            wq_d.ap().rearrange("(c p) f -> p c f", p=128),
        )
        nc.sync.dma_start(selb[:], sel_d.ap())
        nc.sync.dma_start(bds[:], bds_d.ap())
        nc.sync.dma_start(vm[:], vm_d.ap())
        nc.sync.dma_start(
            wo[:].rearrange("p (c e) -> p c e", c=2),
            wo_d.ap().rearrange("(c p) e -> p c e", p=128),
        )
        for e in range(8):
            nc.sync.dma_start(
                xva[:, e * 2048 : (e + 1) * 2048],
                xv_d.ap()[e * 128 : (e + 1) * 128, :],
            )

        # ---- phase 2: v projection (natural layout [s, f]) ----
        with tc.tile_pool(name="psV", bufs=2, space="PSUM") as psvp:
            for sc in range(4):
                pvs = [
                    psvp.tile([128, 256], F32, name=f"pv{sub}", tag=f"psV{sub}")
                    for sub in range(4)
                ]
                for e in range(8):
                    for sub in range(4):
                        nc.tensor.matmul(
                            pvs[sub][:],
                            xva[
                                :,
                                e * 2048 + sc * 512 + sub * 128 : e * 2048
                                + sc * 512
                                + (sub + 1) * 128,
                            ],
                            wv[:, e * F : (e + 1) * F],
                            start=(e == 0),
                            stop=(e == 7),
                        )
                for sub in range(4):
                    nc.scalar.copy(
                        vv[:, sc * 1024 + sub * 256 : sc * 1024 + (sub + 1) * 256],
                        pvs[sub][:],
                    )

        # ---- phase 3: q projection + attention + output projection ----
        xqp = ctx.enter_context(tc.tile_pool(name="xq", bufs=4))
        psSp = ctx.enter_context(tc.tile_pool(name="psS", bufs=5, space="PSUM"))
        flexp = ctx.enter_context(tc.tile_pool(name="flex", bufs=3, space="PSUM"))
        expp = ctx.enter_context(tc.tile_pool(name="expS", bufs=2))
        ptp = ctx.enter_context(tc.tile_pool(name="pt", bufs=4))
        rcpp = ctx.enter_context(tc.tile_pool(name="rcp", bufs=2))
        attp = ctx.enter_context(tc.tile_pool(name="att", bufs=4))
        outp = ctx.enter_context(tc.tile_pool(name="outsb", bufs=2))

        def qproj(sc4):
            pqs = [
                psSp.tile([128, 512], F32, name=f"pq{fold}", tag="psS")
                for fold in range(2)
            ]
            for e2 in range(4):
                xt = xqp.tile([128, 2, 512], BF16, tag="xq")
                nc.sync.dma_start(
                    xt[:],
                    xq_d.ap()[
                        e2 * 256 : (e2 + 1) * 256, sc4 * 512 : (sc4 + 1) * 512
                    ].rearrange("(two p) q -> p two q", p=128),
                )
                for half in range(2):
                    e = 2 * e2 + half
                    for fold in range(2):
                        nc.tensor.matmul(
                            pqs[fold][:],
                            wq[:, e * F + fold * 128 : e * F + fold * 128 + 128],
                            xt[:, half, :],
                            start=(e == 0),
                            stop=(e == 7),
                        )
            for fold in range(2):
                nc.scalar.copy(
                    qT[:, fold * S + sc4 * 512 : fold * S + (sc4 + 1) * 512],
                    pqs[fold][:],
                )

        def outproj(t, atts):
            # atts: [pair0_tile, pair1_tile], each [128, 512] bf16
            for sc2 in range(4):
                ob = outp.tile([128, 1024], F32, tag="outsb")
                for eh in range(2):
                    po = flexp.tile([128, 512], F32, tag="flex")
                    for kc in range(2):
                        nc.tensor.matmul(
                            po[:],
                            atts[kc][:, sc2 * 128 : sc2 * 128 + 128],
                            wo[:, kc * E + eh * 512 : kc * E + eh * 512 + 512],
                            start=(kc == 0),
                            stop=(kc == 1),
                        )
                    nc.vector.tensor_copy(ob[:, eh * 512 : (eh + 1) * 512], po[:])
                row = (4 * t + sc2) * 128
                nc.sync.dma_start(out_d.ap()[row : row + 128, :], ob[:])

        def emit_qk(u, j):
            fold = u.h // 2
            bp = 64 * (u.h % 2)
            c0 = u.lo + 2 * j
            ps = psSp.tile([128, 512], F32, name="ps", tag="psS")
            nc.tensor.matmul(
                ps[:],
                kT[bp : bp + 64, fold * S + c0 * 64 : fold * S + c0 * 64 + 128],
                qT[bp : bp + 64, fold * S + u.t * QS : fold * S + (u.t + 1) * QS],
                start=True,
                stop=True,
            )
            nc.scalar.activation(u.expS[:, j * QS : (j + 1) * QS], ps[:], EXP)

        def emit_denom(u, j):
            nc.tensor.matmul(
                u.accs[0:32, :],
                selb[:, j * 32 : (j + 1) * 32],
                u.expS[:, j * QS : (j + 1) * QS],
                start=(j == 0),
                stop=(j == u.npt - 1),
            )

        def emit_rcp(u):
            rc = rcpp.tile([32, 512], BF16, tag="rcp")
            rs1 = rcpp.tile([32, 512], F32, tag="rcs1")
            rs2 = rcpp.tile([32, 512], F32, tag="rcs2")
            nc.vector.reciprocal_approx_accurate(rs2[:], u.accs[0:32, :], rs1[:])
            nc.vector.tensor_mul(rc[:], rs2[:], vm[:, u.t * QS : (u.t + 1) * QS])
            u.rc = rc
            u.acco = psSp.tile([128, 512], F32, name="acco", tag="psS")

        def emit_bcast_mul(u, j):
            bt = flexp.tile([128, 512], F32, tag="flex")
            nc.tensor.matmul(
                bt[:],
                bds[0 : 2 * u.npt, j * 128 : (j + 1) * 128],
                u.rc[0 : 2 * u.npt, :],
                start=True,
                stop=True,
            )
            pt = ptp.tile([128, 512], BF16, tag="pt")
            nc.vector.tensor_mul(pt[:], u.expS[:, j * QS : (j + 1) * QS], bt[:])
            u.pts.append(pt)

        def emit_av(u, j):
            cp = u.lo // 2 + j
            nc.tensor.matmul(
                u.acco[0:64, :],
                vv[:, cp * F + u.h * 64 : cp * F + u.h * 64 + 64],
                u.pts[j][:],
                start=(j == 0),
                stop=(j == u.npt - 1),
            )

        units = [(t, h) for t in range(T_SLABS) for h in range(HPC)]
        atts_by_t = {t: [] for t in range(T_SLABS)}
        done_by_t = {t: 0 for t in range(T_SLABS)}

        def finish_unit(p):
            if p.half == 0:
                attn_t = attp.tile([128, 512], BF16, tag="att")
                atts_by_t[p.t].append(attn_t)
            else:
                attn_t = atts_by_t[p.t][p.h // 2]
            nc.scalar.copy(attn_t[64 * p.half : 64 * p.half + 64, :], p.acco[0:64, :])
            done_by_t[p.t] += 1
            if done_by_t[p.t] == HPC:
                outproj(p.t, atts_by_t[p.t])

        qproj(0)
        prev = None
        for t, h in units:
            u = _Unit()
            u.t, u.h, u.npt, u.lo, u.half = t, h, NP_T[t], LO[t], h % 2
            u.expS = expp.tile([128, MAXP * QS], BF16, tag="expS")
            u.accs = psSp.tile([128, 512], F32, name="accs", tag="psS")
            u.rc = None
            u.acco = None
            u.pts = []
            u.nd = 0
            if prev is not None:
                emit_rcp(prev)
            nj = max(u.npt, prev.npt if prev is not None else 0)
            for j in range(nj):
                if j < u.npt:
                    emit_qk(u, j)
                if prev is not None and j < prev.npt:
                    emit_bcast_mul(prev, j)
                if prev is not None and j >= AVLAG and j - AVLAG < prev.npt:
                    emit_av(prev, j - AVLAG)
                if j >= DLAG and u.nd < u.npt and u.nd == j - DLAG:
                    emit_denom(u, u.nd)
                    u.nd += 1
            if prev is not None:
                for jj in range(max(nj - AVLAG, 0), prev.npt):
                    emit_av(prev, jj)
                finish_unit(prev)
            while u.nd < u.npt:
                emit_denom(u, u.nd)
                u.nd += 1
            # prefetch next slab's q projection midway through the slab
            if h == 1 and t + 1 < T_SLABS:
                qproj(t + 1)
            prev = u

        # drain the final unit
        emit_rcp(prev)
        for j in range(prev.npt):
            emit_bcast_mul(prev, j)
            if j >= AVLAG:
                emit_av(prev, j - AVLAG)
        for jj in range(max(prev.npt - AVLAG, 0), prev.npt):
            emit_av(prev, jj)
        finish_unit(prev)

    nc.compile()
    return nc


_NC_CACHE = []


def _get_nc():
    if not _NC_CACHE:
        _NC_CACHE.append(build_nc())
    return _NC_CACHE[0]


def _host_consts():
    selc = np.zeros((128, MAXP * 32), np.float32)
    for k in range(128):
        for j in range(MAXP):
            selc[k, j * 32 + 2 * j + k // 64] = 1.0
    bdsel = np.zeros((32, MAXP * 128), np.float32)
    for j in range(MAXP):
        for p in range(128):
            bdsel[2 * j + p // 64, j * 128 + p] = 1.0
    vmask = np.zeros((32, T_SLABS * QS), np.float32)
    for t in range(T_SLABS):
        for m in range(2 * NP_T[t]):
            c = LO[t] + m
            for qb in range(QS // BLK):
                r = 8 * t + qb
                if abs(r - c) <= BAND:
                    vmask[m, t * QS + qb * 64 : t * QS + (qb + 1) * 64] = 1.0
    return selc, bdsel, vmask


def kernel(query, key, value, Wq, Wk, Wv, Wo):
    query = np.asarray(query, np.float32)
    key = np.asarray(key, np.float32)
    value = np.asarray(value, np.float32)
    Wq = np.asarray(Wq, np.float32)
    Wk = np.asarray(Wk, np.float32)
    Wv = np.asarray(Wv, np.float32)
    Wo = np.asarray(Wo, np.float32)

    nc = _get_nc()
    selc, bdsel, vmask = _host_consts()

    in_maps = []
    for c in range(NCORES):
        b, g = divmod(c, HPC)
        fs = slice(F * g, F * (g + 1))
        in_maps.append(
            {
                "xqT": np.ascontiguousarray(query[b].T).astype(BF16NP),
                "xkT": np.ascontiguousarray(key[b].T).astype(BF16NP),
                "xvT": np.ascontiguousarray(value[b].T).astype(BF16NP),
                "wqT": np.ascontiguousarray((Wq[fs, :] * SCALE).T).astype(BF16NP),
                "wkT": np.ascontiguousarray(Wk[fs, :].T).astype(BF16NP),
                "wvT": np.ascontiguousarray(Wv[fs, :].T).astype(BF16NP),
                "woT": np.ascontiguousarray(Wo[:, fs].T).astype(BF16NP),
                "selc": selc.astype(BF16NP),
                "bdsel": bdsel.astype(BF16NP),
                "vmask": vmask,
            }
        )

    res = bass_utils.run_bass_kernel_spmd(nc, in_maps, core_ids=list(range(NCORES)))
    out = np.zeros((B, S, E), np.float32)
    for c in range(NCORES):
        b = c // HPC
        out[b] += res.results[c]["out"]
    return out


# revision 19
# speedup vs baseline: 1.3392x; 1.1437x over previous
"""Block-sparse (banded) attention kernel for Trainium2, 8 NeuronCores.

Sharding: data-parallel over batch (2) x tensor-parallel over heads
(16 heads -> 4 per core).  Each core computes its 4 heads' Q/K/V
projections, banded block attention (|r-c| <= 15 blocks, per-block
softmax), and a partial output projection; the host sums the 4 partial
outputs per batch element.

bf16 matmul pipeline (1 PE cycle/row), fp32 PSUM accumulation.  All
inputs are cast to bf16 on the host and DMA'd over HWDGE, so no device
engine spends time casting.  Attention is software-pipelined at the
column-block-pair level: unit i's QK/softmax-denominator matmuls
interleave with unit i-1's broadcast/AV matmuls, with 2-slot lags on
the cross-engine (exp, rescale) dependencies, so the in-order PE queue
never stalls and the clock stays at the high p-state.

Self-contained: hardcodes all shapes; only needs the concourse tree that
the environment already puts on sys.path.
"""

import sys

for _p in ("/opt/trn_rl_repo",):
    if _p not in sys.path:
        sys.path.insert(0, _p)

from contextlib import ExitStack

import ml_dtypes
import numpy as np

import concourse.bacc as bacc
import concourse.tile as tile
from concourse import bass_utils, mybir

F32 = mybir.dt.float32
BF16 = mybir.dt.bfloat16
EXP = mybir.ActivationFunctionType.Exp
BF16NP = ml_dtypes.bfloat16

B, S, E = 2, 2048, 1024
H, HD, BLK = 16, 64, 64
NB = S // BLK  # 32 blocks
NCORES = 8
HPC = 4  # heads per core
F = HPC * HD  # 256 local features
BAND = 15
SCALE = HD ** -0.5

# per r8-slab (8 query blocks, q=512) column-block ranges, even-extended
T_SLABS = 4
QS = 512  # q extent per slab
LO = []
NP_T = []
for _t in range(T_SLABS):
    lo = max(0, 8 * _t - BAND)
    hi = min(NB - 1, 8 * _t + 7 + BAND)
    if (hi - lo + 1) % 2 == 1:
        if lo > 0:
            lo -= 1
        else:
            hi += 1
    LO.append(lo)
    NP_T.append((hi - lo + 1) // 2)
MAXP = max(NP_T)  # 16 pairs

DLAG = 2  # slots between QK/exp and the denominator matmul
AVLAG = 4  # slots between broadcast/rescale and the AV matmul


class _Unit:
    __slots__ = ("t", "h", "npt", "lo", "expS", "accs", "rc", "acco", "half",
                 "pts", "nd")


def build_nc(debug=False):
    nc = bacc.Bacc("TRN2", target_bir_lowering=False, debug=False)

    xq_d = nc.dram_tensor("xqT", [E, S], BF16, kind="ExternalInput")
    xk_d = nc.dram_tensor("xkT", [E, S], BF16, kind="ExternalInput")
    xv_d = nc.dram_tensor("xvT", [E, S], BF16, kind="ExternalInput")
    wq_d = nc.dram_tensor("wqT", [E, F], BF16, kind="ExternalInput")
    wk_d = nc.dram_tensor("wkT", [E, F], BF16, kind="ExternalInput")
    wv_d = nc.dram_tensor("wvT", [E, F], BF16, kind="ExternalInput")
    wo_d = nc.dram_tensor("woT", [F, E], BF16, kind="ExternalInput")
    sel_d = nc.dram_tensor("selc", [128, MAXP * 32], BF16, kind="ExternalInput")
    bds_d = nc.dram_tensor("bdsel", [32, MAXP * 128], BF16, kind="ExternalInput")
    vm_d = nc.dram_tensor("vmask", [32, T_SLABS * QS], F32, kind="ExternalInput")
    out_d = nc.dram_tensor("out", [S, E], F32, kind="ExternalOutput")

    with tile.TileContext(nc) as tc, ExitStack() as ctx, nc.allow_low_precision(
        reason="bf16 matmul pipeline; fp32 PSUM accumulation"
    ):
        pers = ctx.enter_context(tc.tile_pool(name="pers", bufs=1))
        qT = pers.tile([128, 2 * S], BF16, tag="qT")
        kT = pers.tile([128, 2 * S], BF16, tag="kT")
        vv = pers.tile([128, 16 * F], BF16, tag="vv")
        xva = pers.tile([128, 8 * 2048], BF16, tag="xva")
        wq = pers.tile([128, 8 * F], BF16, tag="wq")
        wk = pers.tile([128, 8 * F], BF16, tag="wk")
        wv = pers.tile([128, 8 * F], BF16, tag="wv")
        wo = pers.tile([128, 2 * E], BF16, tag="wo")
        selb = pers.tile([128, MAXP * 32], BF16, tag="selb")
        bds = pers.tile([32, MAXP * 128], BF16, tag="bds")
        vm = pers.tile([32, T_SLABS * QS], F32, tag="vm")

        # k-projection weights first: phase 1 is on the critical path
        nc.sync.dma_start(
            wk[:].rearrange("p (c f) -> p c f", c=8),
            wk_d.ap().rearrange("(c p) f -> p c f", p=128),
        )

        # ---- phase 1: k projection (kT layout [f, s]) ----
        with tc.tile_pool(name="xk", bufs=3) as xkp, tc.tile_pool(
            name="psK", bufs=1, space="PSUM"
        ) as pskp:
            psK = pskp.tile([128, 4096], F32)
            for e in range(8):
                xt = xkp.tile([128, S], BF16, tag="xk")
                nc.sync.dma_start(xt[:], xk_d.ap()[e * 128 : (e + 1) * 128, :])
                for fold in range(2):
                    for sc in range(4):
                        nc.tensor.matmul(
                            psK[:, (fold * 4 + sc) * 512 : (fold * 4 + sc + 1) * 512],
                            wk[:, e * F + fold * 128 : e * F + fold * 128 + 128],
                            xt[:, sc * 512 : (sc + 1) * 512],
                            start=(e == 0),
                            stop=(e == 7),
                        )
            for fold in range(2):
                for sc in range(4):
                    nc.scalar.copy(
                        kT[:, fold * S + sc * 512 : fold * S + (sc + 1) * 512],
                        psK[:, (fold * 4 + sc) * 512 : (fold * 4 + sc + 1) * 512],
                    )

        # remaining weights/constants + resident xv after the phase-1 loads
        nc.sync.dma_start(
            wv[:].rearrange("p (c f) -> p c f", c=8),
            wv_d.ap().rearrange("(c p) f -> p c f", p=128),
        )
        nc.sync.dma_start(
            wq[:].rearrange("p (c f) -> p c f", c=8),
            wq_d.ap().rearrange("(c p) f -> p c f", p=128),
        )
        nc.sync.dma_start(selb[:], sel_d.ap())
        nc.sync.dma_start(bds[:], bds_d.ap())
        nc.sync.dma_start(vm[:], vm_d.ap())
        nc.sync.dma_start(
            wo[:].rearrange("p (c e) -> p c e", c=2),
            wo_d.ap().rearrange("(c p) e -> p c e", p=128),
        )
        for e in range(8):
            nc.sync.dma_start(
                xva[:, e * 2048 : (e + 1) * 2048],
                xv_d.ap()[e * 128 : (e + 1) * 128, :],
            )

        # ---- phase 2: v projection (natural layout [s, f]) ----
        with tc.tile_pool(name="psV", bufs=2, space="PSUM") as psvp:
            for sc in range(4):
                pvs = [
                    psvp.tile([128, 256], F32, name=f"pv{sub}", tag=f"psV{sub}")
                    for sub in range(4)
                ]
                for e in range(8):
                    for sub in range(4):
                        nc.tensor.matmul(
                            pvs[sub][:],
                            xva[
                                :,
                                e * 2048 + sc * 512 + sub * 128 : e * 2048
                                + sc * 512
                                + (sub + 1) * 128,
                            ],
                            wv[:, e * F : (e + 1) * F],
                            start=(e == 0),
                            stop=(e == 7),
                        )
                for sub in range(4):
                    nc.scalar.copy(
                        vv[:, sc * 1024 + sub * 256 : sc * 1024 + (sub + 1) * 256],
                        pvs[sub][:],
                    )

        # ---- phase 3: q projection + attention + output projection ----
        xqp = ctx.enter_context(tc.tile_pool(name="xq", bufs=4))
        psSp = ctx.enter_context(tc.tile_pool(name="psS", bufs=5, space="PSUM"))
        flexp = ctx.enter_context(tc.tile_pool(name="flex", bufs=3, space="PSUM"))
        expp = ctx.enter_context(tc.tile_pool(name="expS", bufs=2))
        ptp = ctx.enter_context(tc.tile_pool(name="pt", bufs=5))
        rcpp = ctx.enter_context(tc.tile_pool(name="rcp", bufs=2))
        attp = ctx.enter_context(tc.tile_pool(name="att", bufs=4))
        outp = ctx.enter_context(tc.tile_pool(name="outsb", bufs=2))

        def qproj(sc4):
            pqs = [
                psSp.tile([128, 512], F32, name=f"pq{fold}", tag="psS")
                for fold in range(2)
            ]
            for e2 in range(4):
                xt = xqp.tile([128, 2, 512], BF16, tag="xq")
                nc.sync.dma_start(
                    xt[:],
                    xq_d.ap()[
                        e2 * 256 : (e2 + 1) * 256, sc4 * 512 : (sc4 + 1) * 512
                    ].rearrange("(two p) q -> p two q", p=128),
                )
                for half in range(2):
                    e = 2 * e2 + half
                    for fold in range(2):
                        nc.tensor.matmul(
                            pqs[fold][:],
                            wq[:, e * F + fold * 128 : e * F + fold * 128 + 128],
                            xt[:, half, :],
                            start=(e == 0),
                            stop=(e == 7),
                        )
            for fold in range(2):
                nc.scalar.copy(
                    qT[:, fold * S + sc4 * 512 : fold * S + (sc4 + 1) * 512],
                    pqs[fold][:],
                )

        def outproj(t, atts):
            # atts: [pair0_tile, pair1_tile], each [128, 512] bf16
            for sc2 in range(4):
                ob = outp.tile([128, 1024], F32, tag="outsb")
                for eh in range(2):
                    po = flexp.tile([128, 512], F32, tag="flex")
                    for kc in range(2):
                        nc.tensor.matmul(
                            po[:],
                            atts[kc][:, sc2 * 128 : sc2 * 128 + 128],
                            wo[:, kc * E + eh * 512 : kc * E + eh * 512 + 512],
                            start=(kc == 0),
                            stop=(kc == 1),
                        )
                    nc.vector.tensor_copy(ob[:, eh * 512 : (eh + 1) * 512], po[:])
                row = (4 * t + sc2) * 128
                nc.sync.dma_start(out_d.ap()[row : row + 128, :], ob[:])

        def emit_qk(u, j):
            fold = u.h // 2
            bp = 64 * (u.h % 2)
            c0 = u.lo + 2 * j
            ps = psSp.tile([128, 512], F32, name="ps", tag="psS")
            nc.tensor.matmul(
                ps[:],
                kT[bp : bp + 64, fold * S + c0 * 64 : fold * S + c0 * 64 + 128],
                qT[bp : bp + 64, fold * S + u.t * QS : fold * S + (u.t + 1) * QS],
                start=True,
                stop=True,
            )
            nc.scalar.activation(u.expS[:, j * QS : (j + 1) * QS], ps[:], EXP)

        def emit_denom(u, j):
            nc.tensor.matmul(
                u.accs[0:32, :],
                selb[:, j * 32 : (j + 1) * 32],
                u.expS[:, j * QS : (j + 1) * QS],
                start=(j == 0),
                stop=(j == u.npt - 1),
            )

        def emit_rcp(u):
            rc = rcpp.tile([32, 512], BF16, tag="rcp")
            rs1 = rcpp.tile([32, 512], F32, tag="rcs1")
            rs2 = rcpp.tile([32, 512], F32, tag="rcs2")
            nc.vector.reciprocal_approx_accurate(rs2[:], u.accs[0:32, :], rs1[:])
            nc.vector.tensor_mul(rc[:], rs2[:], vm[:, u.t * QS : (u.t + 1) * QS])
            u.rc = rc
            u.acco = psSp.tile([128, 512], F32, name="acco", tag="psS")

        def emit_bcast_mul(u, j):
            bt = flexp.tile([128, 512], F32, tag="flex")
            nc.tensor.matmul(
                bt[:],
                bds[0 : 2 * u.npt, j * 128 : (j + 1) * 128],
                u.rc[0 : 2 * u.npt, :],
                start=True,
                stop=True,
            )
            pt = ptp.tile([128, 512], BF16, tag="pt")
            nc.vector.tensor_mul(pt[:], u.expS[:, j * QS : (j + 1) * QS], bt[:])
            u.pts.append(pt)

        def emit_av(u, j):
            cp = u.lo // 2 + j
            nc.tensor.matmul(
                u.acco[0:64, :],
                vv[:, cp * F + u.h * 64 : cp * F + u.h * 64 + 64],
                u.pts[j][:],
                start=(j == 0),
                stop=(j == u.npt - 1),
            )

        units = [(t, h) for t in range(T_SLABS) for h in range(HPC)]
        atts_by_t = {t: [] for t in range(T_SLABS)}
        done_by_t = {t: 0 for t in range(T_SLABS)}

        def finish_unit(p):
            if p.half == 0:
                attn_t = attp.tile([128, 512], BF16, tag="att")
                atts_by_t[p.t].append(attn_t)
            else:
                attn_t = atts_by_t[p.t][p.h // 2]
            nc.scalar.copy(attn_t[64 * p.half : 64 * p.half + 64, :], p.acco[0:64, :])
            done_by_t[p.t] += 1
            if done_by_t[p.t] == HPC:
                outproj(p.t, atts_by_t[p.t])

        qproj(0)
        prev = None
        for t, h in units:
            u = _Unit()
            u.t, u.h, u.npt, u.lo, u.half = t, h, NP_T[t], LO[t], h % 2
            u.expS = expp.tile([128, MAXP * QS], BF16, tag="expS")
            u.accs = psSp.tile([128, 512], F32, name="accs", tag="psS")
            u.rc = None
            u.acco = None
            u.pts = []
            u.nd = 0
            if prev is not None:
                emit_rcp(prev)
            nj = max(u.npt, prev.npt if prev is not None else 0)
            for j in range(nj):
                if j < u.npt:
                    emit_qk(u, j)
                if prev is not None and j < prev.npt:
                    emit_bcast_mul(prev, j)
                if prev is not None and j >= AVLAG and j - AVLAG < prev.npt:
                    emit_av(prev, j - AVLAG)
                if j >= DLAG and u.nd < u.npt and u.nd == j - DLAG:
                    emit_denom(u, u.nd)
                    u.nd += 1
            if prev is not None:
                for jj in range(max(nj - AVLAG, 0), prev.npt):
                    emit_av(prev, jj)
                finish_unit(prev)
            while u.nd < u.npt:
                emit_denom(u, u.nd)
                u.nd += 1
            # prefetch next slab's q projection midway through the slab
            if h == 1 and t + 1 < T_SLABS:
                qproj(t + 1)
            prev = u

        # drain the final unit
        emit_rcp(prev)
        for j in range(prev.npt):
            emit_bcast_mul(prev, j)
            if j >= AVLAG:
                emit_av(prev, j - AVLAG)
        for jj in range(max(prev.npt - AVLAG, 0), prev.npt):
            emit_av(prev, jj)
        finish_unit(prev)

    nc.compile()
    return nc


_NC_CACHE = []


def _get_nc():
    if not _NC_CACHE:
        _NC_CACHE.append(build_nc())
    return _NC_CACHE[0]


def _host_consts():
    selc = np.zeros((128, MAXP * 32), np.float32)
    for k in range(128):
        for j in range(MAXP):
            selc[k, j * 32 + 2 * j + k // 64] = 1.0
    bdsel = np.zeros((32, MAXP * 128), np.float32)
    for j in range(MAXP):
        for p in range(128):
            bdsel[2 * j + p // 64, j * 128 + p] = 1.0
    vmask = np.zeros((32, T_SLABS * QS), np.float32)
    for t in range(T_SLABS):
        for m in range(2 * NP_T[t]):
            c = LO[t] + m
            for qb in range(QS // BLK):
                r = 8 * t + qb
                if abs(r - c) <= BAND:
                    vmask[m, t * QS + qb * 64 : t * QS + (qb + 1) * 64] = 1.0
    return selc, bdsel, vmask


def kernel(query, key, value, Wq, Wk, Wv, Wo):
    query = np.asarray(query, np.float32)
    key = np.asarray(key, np.float32)
    value = np.asarray(value, np.float32)
    Wq = np.asarray(Wq, np.float32)
    Wk = np.asarray(Wk, np.float32)
    Wv = np.asarray(Wv, np.float32)
    Wo = np.asarray(Wo, np.float32)

    nc = _get_nc()
    selc, bdsel, vmask = _host_consts()

    in_maps = []
    for c in range(NCORES):
        b, g = divmod(c, HPC)
        fs = slice(F * g, F * (g + 1))
        in_maps.append(
            {
                "xqT": np.ascontiguousarray(query[b].T).astype(BF16NP),
                "xkT": np.ascontiguousarray(key[b].T).astype(BF16NP),
                "xvT": np.ascontiguousarray(value[b].T).astype(BF16NP),
                "wqT": np.ascontiguousarray((Wq[fs, :] * SCALE).T).astype(BF16NP),
                "wkT": np.ascontiguousarray(Wk[fs, :].T).astype(BF16NP),
                "wvT": np.ascontiguousarray(Wv[fs, :].T).astype(BF16NP),
                "woT": np.ascontiguousarray(Wo[:, fs].T).astype(BF16NP),
                "selc": selc.astype(BF16NP),
                "bdsel": bdsel.astype(BF16NP),
                "vmask": vmask,
            }
        )

    res = bass_utils.run_bass_kernel_spmd(nc, in_maps, core_ids=list(range(NCORES)))
    out = np.zeros((B, S, E), np.float32)
    for c in range(NCORES):
        b = c // HPC
        out[b] += res.results[c]["out"]
    return out
